# revision 29
# baseline (speedup 1.0000x reference)
"""Neighbor-slice attention (nn_AttentionModule) on 8 TRN2 NeuronCores.

Per core: 2 of 16 slices + 1 halo slice each side, packed by the host.
Key performance ideas (v2, from ~193us baseline):

- The f=k^T q matmul contracts over only ci=64 of 128 partitions.  Instead
  of zero-padding to 128 (the baseline), the two neighbor sides' k are
  PACKED into one tile (side0 at partitions 0:64, side1 at 64:128) and the
  two f matmuls run CONCURRENTLY as row-tiled 64-deep matmuls
  (tile_position (0,0)/(64,0), auto-derived from base partitions).  This
  halves PE time for f.  q is duplicated to both partition halves so each
  tile's rhs streams from its own partitions.
- The two sides' f outputs land in two banks of a single 5-bank rotating
  PSUM tensor, so ONE exp instruction covers both sides via a strided
  2-chunk AP: halves per-instruction overhead on ACT/DVE (the exp engines
  are the co-bottleneck).  exp alternates ACT (true exp) / DVE
  (Schraudolph bf16 fast-exp) per j.  The 5-bank rotation gives a ~2.4
  slot reuse distance so the f stream never stalls on exp completion;
  zps allocations are injected into the same rotation.
- The k-projection bias bk mathematically cancels in softmax (it shifts
  every logit of a query by the same constant), so the k/v evacuations are
  pure casts and ride CASTING DMAs issued from gpsimd (software DGE)
  instead of burning DVE/ACT cycles.
- vg carries the ones-column at col 0 + v at cols 1:65 (no zero half), so
  yps is [65, w]: partition 0 = softmax denominator, 1:65 = y.
- The two z matmuls collapse into ONE dense 128-deep matmul with
  wz2 = [Wz^T; Wz^T] against ysb_st = [y0/d0 ; y1/d1], plus the identity
  residual matmul whose rhs is xr = x + c2 (c2 = 2(Wz bv + bz) folded on
  the host) -- so the output DMAs straight from PSUM, no evacuation op.
- Per q-block the PE slot for j is: fA_j, fB_j (concurrent row-tiled
  pair), y0_{j-2}, y1_{j-2}; exp_j issues right after fB_j; the two-j y
  skew gives each merged exp ~1.3us of cover.  Unit tails
  (yc/recip/bcast/mul/z/out-dma) are deferred into later units' slots so
  the strict-FIFO ACT/DVE queues never head-of-line block the PE.
"""

import sys

for _p in ("/opt/trn_rl_repo",):
    if _p not in sys.path:
        sys.path.insert(0, _p)

import numpy as np

N_FULL, C, H, W = 16, 128, 48, 48
HW = H * W            # 2304
CI = C // 2           # 64
KC = HW // 128        # 18 k-chunks per slice
NCORES = 8
NLOC = N_FULL // NCORES  # 2 local slices per core

# q-blocks (start, width); width <= 512 (one PSUM bank)
QBS = [(0, 512), (512, 512), (1024, 512), (1536, 512), (2048, 256)]

# bf16 Schraudolph constants: bits16 = round(x*log2(e)*128 + B16)
_S16 = 184.66496736235803          # 2**7 / ln(2)
_B16 = 16256.0 - 4.75              # 127*2**7 with mid-sawtooth correction

NBANK = 5             # f/z rotating PSUM banks
BW = 512              # bank width in f32

_NC_CACHE = {}
LAST_RESULTS = None
TRACE = False


def _build_nc():
    import concourse.bass as bass
    import concourse.mybir as mybir
    import concourse.tile as tile
    from concourse import bacc

    f32 = mybir.dt.float32
    bf16 = mybir.dt.bfloat16
    f16 = mybir.dt.float16
    i16 = mybir.dt.int16
    FT = mybir.ActivationFunctionType

    nc = bacc.Bacc()

    xh_d = nc.declare_dram_parameter("xh", [4, C, HW], f16, isOutput=False)
    xr_d = nc.declare_dram_parameter("xr", [NLOC, C, HW], f16, isOutput=False)
    wqk_d = nc.declare_dram_parameter("wqk", [C, C], f16, isOutput=False)
    wv_d = nc.declare_dram_parameter("wv", [C, CI], f16, isOutput=False)
    wz2_d = nc.declare_dram_parameter("wz2", [C, C], bf16, isOutput=False)
    bq_d = nc.declare_dram_parameter("bq", [CI, 1], f32, isOutput=False)
    ident_d = nc.declare_dram_parameter("ident", [C, C], f16, isOutput=False)
    vgp_d = nc.declare_dram_parameter("vgp", [C, KC, CI], bf16,
                                      isOutput=False)
    out_d = nc.declare_dram_parameter("out", [NLOC, C, HW], f32, isOutput=True)

    # vg cols per chunk: 0 = ones, 1:64 = zeros, 64:128 = v.  The softmax
    # denominator then lands on yps partition 0 and y on 64:128, keeping
    # every engine partition shift 64-aligned (walrus requirement).
    VW = C

    with tile.TileContext(nc) as tc:
        with tc.tile_pool(name="const", bufs=1) as cpool, \
             tc.tile_pool(name="xb", bufs=4) as xbpool, \
             tc.tile_pool(name="xr", bufs=2) as xrpool, \
             tc.tile_pool(name="qt", bufs=2) as qtpool, \
             tc.tile_pool(name="kt", bufs=2) as ktpool, \
             tc.tile_pool(name="vg", bufs=4) as vgpool, \
             tc.tile_pool(name="at", bufs=8) as atpool, \
             tc.tile_pool(name="yc", bufs=4) as ycpool, \
             tc.tile_pool(name="rr", bufs=4) as rrpool, \
             tc.tile_pool(name="rb", bufs=4) as rbpool, \
             tc.tile_pool(name="ysb", bufs=5) as ysbpool, \
             tc.tile_pool(name="osb", bufs=3) as opool:

            # ---- constants ----
            wqk_t = cpool.tile([C, C], f16, tag="wqk")
            wv_t = cpool.tile([C, CI], f16, tag="wv")
            wz2_t = cpool.tile([C, C], bf16, tag="wz2")
            bq_t = cpool.tile([CI, 1], f32, tag="bq")
            id_t = cpool.tile([C, C], f16, tag="id")

            xb_t = [xbpool.tile([C, HW], f16, tag="xb", name=f"xb{s}")
                    for s in range(4)]
            xr_t = [xrpool.tile([C, HW], f16, tag="xr", name=f"xr{s}")
                    for s in range(NLOC)]
            # ktpair[p]: partitions 0:64 = k of slice p, 64:128 = slice p+2
            kt_t = [ktpool.tile([C, HW], f16, tag="kt", name=f"ktp{p}")
                    for p in range(2)]
            # qt2[n]: q of local slice n duplicated into both halves
            qt_t = [qtpool.tile([C, HW], f16, tag="qt", name=f"qt{n}")
                    for n in range(NLOC)]
            vg_t = [vgpool.tile([C, KC, VW], bf16, tag="vg", name=f"vg{s}")
                    for s in range(4)]

            # ---- load order: projection critical path first ----
            nc.sync.dma_start(out=wqk_t, in_=wqk_d[:, :])
            nc.sync.dma_start(out=xb_t[0], in_=xh_d[0])
            nc.sync.dma_start(out=wv_t, in_=wv_d[:, :])
            nc.sync.dma_start(out=bq_t, in_=bq_d[:, :])
            nc.sync.dma_start(out=vg_t[0][:, :, 0:CI], in_=vgp_d[:, :, :])
            nc.sync.dma_start(out=xb_t[1], in_=xh_d[1])
            nc.sync.dma_start(out=vg_t[1][:, :, 0:CI], in_=vgp_d[:, :, :])
            nc.sync.dma_start(out=xb_t[2], in_=xh_d[2])
            nc.sync.dma_start(out=vg_t[2][:, :, 0:CI], in_=vgp_d[:, :, :])
            nc.sync.dma_start(out=xb_t[3], in_=xh_d[3])
            nc.sync.dma_start(out=vg_t[3][:, :, 0:CI], in_=vgp_d[:, :, :])
            nc.sync.dma_start(out=wz2_t, in_=wz2_d[:, :])
            nc.sync.dma_start(out=id_t, in_=ident_d[:, :])
            for n in range(NLOC):
                nc.sync.dma_start(out=xr_t[n], in_=xr_d[n])

            # ---- projections; evacuations ride casting DMAs (gpsimd swdge)
            # kt bias bk cancels in softmax -> pure cast; q keeps bias bq
            # (ACT) and is duplicated to partitions 64:128 via SBUF DMA.
            T3 = HW // 3             # 768
            KH = KC // 2
            with tc.tile_pool(name="pp", bufs=2, space="PSUM") as pp, \
                 tc.tile_pool(name="pv", bufs=2, space="PSUM") as pv:
                for s in range(4):
                    # kt destination: pair p = s % 2, half = s // 2
                    ktp = kt_t[s % 2]
                    klo = CI * (s // 2)
                    vg = vg_t[s]
                    pvt_h = [None, None]

                    def v_chunk(j, s=s, vg=vg, pvt_h=pvt_h):
                        h, jj = divmod(j, KH)
                        if jj == 0:
                            pvt_h[h] = pv.tile([C, KH * CI], f32, tag="pv",
                                               name=f"pvt{h}")
                        nc.tensor.matmul(
                            pvt_h[h][:, CI * jj:CI * (jj + 1)],
                            lhsT=xb_t[s][:, 128 * j:128 * (j + 1)],
                            rhs=wv_t, start=True, stop=True)
                        if jj == KH - 1:
                            dst = vg[:, KH * h:KH * (h + 1), CI:2 * CI]
                            rsrc = pvt_h[h].rearrange("p (j d) -> p j d",
                                                      d=CI)
                            if h == 0:
                                nc.scalar.activation(dst, rsrc, FT.Copy)
                            else:
                                nc.vector.tensor_copy(dst, rsrc)

                    vidx = 0
                    for t in range(3):
                        t0 = T3 * t
                        pq = pp.tile([C, T3], f32, tag="pp")
                        for (b0, bw) in ((0, 512), (512, 256)):
                            nc.tensor.matmul(pq[:, b0:b0 + bw], lhsT=wqk_t,
                                             rhs=xb_t[s][:, t0 + b0:t0 + b0 + bw],
                                             start=True, stop=True)
                        for _ in range(6):
                            v_chunk(vidx)
                            vidx += 1
                        # k: cast-only copy into the packed pair tile
                        # (bias bk cancels in softmax); split ACT/DVE
                        kdst = ktp[klo:klo + CI, t0:t0 + T3]
                        if (3 * s + t) % 4 == 0:
                            nc.scalar.activation(kdst, pq[CI:C, :], FT.Copy)
                        else:
                            nc.vector.tensor_copy(kdst, pq[CI:C, :])
                        if s in (1, 2):
                            qt = qt_t[s - 1]
                            nc.scalar.activation(qt[0:CI, t0:t0 + T3],
                                                 pq[0:CI, :], FT.Identity,
                                                 bias=bq_t)
                            nc.sync.dma_start(
                                out=qt[CI:C, t0:t0 + T3],
                                in_=qt[0:CI, t0:t0 + T3])

            # ---- attention ----
            # PSUM: 3 pair tiles ([128,1024] = 2 banks each; fA in bank 0,
            # fB in bank 1, exp reads the contiguous span) + 2 yps banks.
            # zps borrows a pair-pool slot.  Separate pool tensors keep the
            # dependency tracker precise (strided APs over one big tensor
            # go conservative and serialize the PE stream).
            with tc.tile_pool(name="pf", bufs=3, space="PSUM") as pf, \
                 tc.tile_pool(name="py", bufs=2, space="PSUM") as py:

                units = [(n, qi) for n in range(NLOC)
                         for qi in range(len(QBS))]
                pend = {}          # slot_idx -> [fn]
                slotbox = [0]
                eidx = [0]

                def fire_pend():
                    s = slotbox[0]
                    if s in pend:
                        for fn in pend.pop(s):
                            fn()

                def sched(ds, fn):
                    pend.setdefault(slotbox[0] + ds, []).append(fn)

                for ui, (n, qi) in enumerate(units):
                    q0, w = QBS[qi]
                    st = {"yps": [None, None], "ft": {}, "at": {}}

                    def fpair(j, n=n, q0=q0, w=w, st=st):
                        ft = pf.tile([C, 1024], f32, tag="ft", name="ftp")
                        st["ft"][j] = ft
                        nc.tensor.matmul(
                            ft[:, 0:w],
                            lhsT=kt_t[n][0:CI, 128 * j:128 * (j + 1)],
                            rhs=qt_t[n][0:CI, q0:q0 + w],
                            start=True, stop=True)
                        nc.tensor.matmul(
                            ft[:, BW:BW + w],
                            lhsT=kt_t[n][CI:C, 128 * j:128 * (j + 1)],
                            rhs=qt_t[n][CI:C, q0:q0 + w],
                            start=True, stop=True)

                    def expf(j, st=st, w=w):
                        ft = st["ft"].pop(j)
                        span = BW + w
                        at = atpool.tile([C, 1024], bf16, tag="at",
                                         name="atp")
                        # 10 exps on ACT (true exp), 8 on DVE (fast-exp)
                        if j % 2 == 0 or j == KC - 1:
                            nc.scalar.activation(at[:, 0:span],
                                                 ft[:, 0:span], FT.Exp)
                        else:
                            nc.vector.tensor_scalar(
                                at.bitcast(i16)[:, 0:span], ft[:, 0:span],
                                _S16, _B16,
                                op0=mybir.AluOpType.mult,
                                op1=mybir.AluOpType.add)
                        st["at"][j] = at

                    def ypair(j, n=n, w=w, st=st):
                        first = (j == 0)
                        last = (j == KC - 1)
                        if first:
                            for side in range(2):
                                yt = py.tile([C, BW], f32, tag="py",
                                             name=f"yps{side}")
                                st["yps"][side] = yt
                        at = st["at"].pop(j)
                        for side in range(2):
                            kv = n + 2 * side
                            nc.tensor.matmul(
                                st["yps"][side][:, 0:w],
                                lhsT=vg_t[kv][:, j, :],
                                rhs=at[:, BW * side:BW * side + w],
                                start=first, stop=last)

                    # ---- tail ops for THIS unit, deferred ----
                    yc = [None, None]
                    rr = [None, None]
                    rb = [None, None]
                    ysb = [None]
                    zbox = [None]

                    def yc_copy(side, st=st, w=w, yc=yc):
                        t = ycpool.tile([C, BW], f32, tag="yc",
                                        name=f"yc{side}")
                        yc[side] = t
                        if side == 0:
                            nc.scalar.activation(
                                t[:, 0:w], st["yps"][side][:, 0:w],
                                FT.Copy)
                        else:
                            nc.vector.tensor_copy(
                                t[:, 0:w], st["yps"][side][:, 0:w])

                    def recip(side, w=w, yc=yc, rr=rr):
                        t = rrpool.tile([1, BW], f32, tag="rr",
                                        name=f"rr{side}")
                        rr[side] = t
                        nc.vector.reciprocal_approx_fast(
                            t[:, 0:w], yc[side][0:1, 0:w])

                    def bcast(side, w=w, rr=rr, rb=rb):
                        t = rbpool.tile([C, BW], f32, tag="rb",
                                        name=f"rb{side}")
                        rb[side] = t
                        nc.gpsimd.partition_broadcast(t[:, 0:w],
                                                      rr[side][:, 0:w])

                    def mul(side, w=w, yc=yc, rb=rb, ysb=ysb):
                        # all-SBUF on gpsimd; 64-aligned partition shift
                        if ysb[0] is None:
                            ysb[0] = ysbpool.tile([C, BW], bf16,
                                                  tag="ysb", name="ysb")
                        nc.gpsimd.tensor_mul(
                            ysb[0][CI * side:CI * (side + 1), 0:w],
                            yc[side][CI:C, 0:w],
                            rb[side][CI:C, 0:w])

                    def zout(n=n, q0=q0, w=w, ysb=ysb, zbox=zbox):
                        zt = pf.tile([C, 1024], f32, tag="ft", name="zps")
                        zps = zt[:, 0:w]
                        nc.tensor.matmul(zps, lhsT=wz2_t,
                                         rhs=ysb[0][:, 0:w],
                                         start=True, stop=False)
                        nc.tensor.matmul(zps, lhsT=id_t,
                                         rhs=xr_t[n][:, q0:q0 + w],
                                         start=False, stop=True)
                        # DMA can't read PSUM: split-column evac on
                        # DVE+ACT so the z slot frees fast, then DMA
                        osb = opool.tile([C, BW], f32, tag="osb",
                                         name="osb")
                        h2 = w // 2
                        nc.vector.tensor_copy(osb[:, 0:h2], zt[:, 0:h2])
                        nc.scalar.activation(osb[:, h2:w], zt[:, h2:w],
                                             FT.Copy)
                        nc.sync.dma_start(out=out_d[n][:, q0:q0 + w],
                                          in_=osb[:, 0:w])

                    # ---- emit this unit's slots (y lags f by 3 slots so
                    # the merged exp latency is fully covered) ----
                    for j in range(KC):
                        fpair(j)
                        expf(j)
                        fire_pend()
                        if j >= 3:
                            ypair(j - 3)
                        slotbox[0] += 1

                    # cross-boundary: last three y pairs + evacuations
                    # (bind fns via defaults -- loop scope names rebind!)
                    sched(0, lambda f=ypair: f(KC - 3))
                    sched(1, lambda f=ypair: f(KC - 2))
                    sched(2, lambda f=ypair: f(KC - 1))
                    sched(3, lambda f=yc_copy: f(0))
                    sched(3, lambda f=yc_copy: f(1))
                    sched(5, lambda f=recip: f(0))
                    sched(6, lambda f=recip: f(1))
                    sched(8, lambda f=bcast: f(0))
                    sched(9, lambda f=bcast: f(1))
                    sched(11, lambda f=mul: f(0))
                    sched(12, lambda f=mul: f(1))
                    # z flushes THREE units later: its matmul sits in the
                    # strict-FIFO PE queue, so it must never wait on the
                    # (engine-hopping) ysb chain or it blocks the stream
                    sched(52, zout)

                # drain remaining pend slots (no more f work)
                while pend:
                    fire_pend()
                    slotbox[0] += 1

    nc.compile()
    return nc


def _get_nc():
    if "nc" not in _NC_CACHE:
        _NC_CACHE["nc"] = _build_nc()
    return _NC_CACHE["nc"]


def _host_inputs(features, Wq, bq, Wk, bk, Wv, bv, Wz, bz):
    import ml_dtypes
    X = np.ascontiguousarray(
        np.asarray(features, np.float32).reshape(N_FULL, C, HW))
    wqk = np.ascontiguousarray(
        np.concatenate([Wq.T, Wk.T], axis=1).astype(np.float16))
    wv = np.ascontiguousarray(np.asarray(Wv).T.astype(np.float16))
    bqv = np.asarray(bq, np.float32).reshape(CI, 1)
    c2 = (2.0 * (np.asarray(Wz) @ np.asarray(bv) + np.asarray(bz))).astype(
        np.float32)
    ident = np.eye(C, dtype=np.float16)
    wzT = np.asarray(Wz).T.astype(np.float32)
    wz2 = np.ascontiguousarray(
        np.concatenate([wzT, wzT], axis=0).astype(ml_dtypes.bfloat16))
    vgp = np.zeros((C, KC, CI), ml_dtypes.bfloat16)
    vgp[:, :, 0] = 1.0
    in_maps = []
    for i in range(NCORES):
        idx = [max(2 * i - 1, 0), 2 * i, 2 * i + 1, min(2 * i + 2, N_FULL - 1)]
        xr = (X[2 * i:2 * i + 2] + c2[None, :, None]).astype(np.float16)
        in_maps.append({
            "xh": np.ascontiguousarray(X[idx].astype(np.float16)),
            "xr": np.ascontiguousarray(xr),
            "wqk": wqk, "wv": wv, "wz2": wz2, "bq": bqv,
            "ident": ident, "vgp": vgp,
        })
    return in_maps


def kernel(features, Wq, bq, Wk, bk, Wv, bv, Wz, bz):
    global LAST_RESULTS
    from concourse.bass_utils import run_bass_kernel_spmd

    nc = _get_nc()
    in_maps = _host_inputs(features, Wq, bq, Wk, bk, Wv, bv, Wz, bz)
    res = run_bass_kernel_spmd(nc, in_maps, core_ids=list(range(NCORES)),
                               trace=TRACE)
    LAST_RESULTS = res
    out = np.empty((N_FULL, C, H, W), np.float32)
    for i in range(NCORES):
        out[2 * i:2 * i + 2] = res.results[i]["out"].reshape(NLOC, C, H, W)
    return out


# revision 30
# speedup vs baseline: 1.0017x; 1.0017x over previous
"""Neighbor-slice attention (nn_AttentionModule) on 8 TRN2 NeuronCores.

Per core: 2 of 16 slices + 1 halo slice each side, packed by the host.
Key performance ideas (v2, from ~193us baseline):

- The f=k^T q matmul contracts over only ci=64 of 128 partitions.  Instead
  of zero-padding to 128 (the baseline), the two neighbor sides' k are
  PACKED into one tile (side0 at partitions 0:64, side1 at 64:128) and the
  two f matmuls run CONCURRENTLY as row-tiled 64-deep matmuls
  (tile_position (0,0)/(64,0), auto-derived from base partitions).  This
  halves PE time for f.  q is duplicated to both partition halves so each
  tile's rhs streams from its own partitions.
- The two sides' f outputs land in two banks of a single 5-bank rotating
  PSUM tensor, so ONE exp instruction covers both sides via a strided
  2-chunk AP: halves per-instruction overhead on ACT/DVE (the exp engines
  are the co-bottleneck).  exp alternates ACT (true exp) / DVE
  (Schraudolph bf16 fast-exp) per j.  The 5-bank rotation gives a ~2.4
  slot reuse distance so the f stream never stalls on exp completion;
  zps allocations are injected into the same rotation.
- The k-projection bias bk mathematically cancels in softmax (it shifts
  every logit of a query by the same constant), so the k/v evacuations are
  pure casts and ride CASTING DMAs issued from gpsimd (software DGE)
  instead of burning DVE/ACT cycles.
- vg carries the ones-column at col 0 + v at cols 1:65 (no zero half), so
  yps is [65, w]: partition 0 = softmax denominator, 1:65 = y.
- The two z matmuls collapse into ONE dense 128-deep matmul with
  wz2 = [Wz^T; Wz^T] against ysb_st = [y0/d0 ; y1/d1], plus the identity
  residual matmul whose rhs is xr = x + c2 (c2 = 2(Wz bv + bz) folded on
  the host) -- so the output DMAs straight from PSUM, no evacuation op.
- Per q-block the PE slot for j is: fA_j, fB_j (concurrent row-tiled
  pair), y0_{j-2}, y1_{j-2}; exp_j issues right after fB_j; the two-j y
  skew gives each merged exp ~1.3us of cover.  Unit tails
  (yc/recip/bcast/mul/z/out-dma) are deferred into later units' slots so
  the strict-FIFO ACT/DVE queues never head-of-line block the PE.
"""

import sys

for _p in ("/opt/trn_rl_repo",):
    if _p not in sys.path:
        sys.path.insert(0, _p)

import numpy as np

N_FULL, C, H, W = 16, 128, 48, 48
HW = H * W            # 2304
CI = C // 2           # 64
KC = HW // 128        # 18 k-chunks per slice
NCORES = 8
NLOC = N_FULL // NCORES  # 2 local slices per core

# q-blocks (start, width); width <= 512 (one PSUM bank)
QBS = [(0, 512), (512, 512), (1024, 512), (1536, 512), (2048, 256)]

# bf16 Schraudolph constants: bits16 = round(x*log2(e)*128 + B16)
_S16 = 184.66496736235803          # 2**7 / ln(2)
_B16 = 16256.0 - 4.75              # 127*2**7 with mid-sawtooth correction

NBANK = 5             # f/z rotating PSUM banks
BW = 512              # bank width in f32

_NC_CACHE = {}
LAST_RESULTS = None
TRACE = False


def _build_nc():
    import concourse.bass as bass
    import concourse.mybir as mybir
    import concourse.tile as tile
    from concourse import bacc

    f32 = mybir.dt.float32
    bf16 = mybir.dt.bfloat16
    f16 = mybir.dt.float16
    i16 = mybir.dt.int16
    FT = mybir.ActivationFunctionType

    nc = bacc.Bacc()

    xh_d = nc.declare_dram_parameter("xh", [4, C, HW], f16, isOutput=False)
    xr_d = nc.declare_dram_parameter("xr", [NLOC, C, HW], f16, isOutput=False)
    wqk_d = nc.declare_dram_parameter("wqk", [C, C], f16, isOutput=False)
    wv_d = nc.declare_dram_parameter("wv", [C, CI], f16, isOutput=False)
    wz2_d = nc.declare_dram_parameter("wz2", [C, C], bf16, isOutput=False)
    bq_d = nc.declare_dram_parameter("bq", [CI, 1], f32, isOutput=False)
    ident_d = nc.declare_dram_parameter("ident", [C, C], f16, isOutput=False)
    vgp_d = nc.declare_dram_parameter("vgp", [C, KC, CI], bf16,
                                      isOutput=False)
    out_d = nc.declare_dram_parameter("out", [NLOC, C, HW], f32, isOutput=True)

    # vg cols per chunk: 0 = ones, 1:64 = zeros, 64:128 = v.  The softmax
    # denominator then lands on yps partition 0 and y on 64:128, keeping
    # every engine partition shift 64-aligned (walrus requirement).
    VW = C

    with tile.TileContext(nc) as tc:
        with tc.tile_pool(name="const", bufs=1) as cpool, \
             tc.tile_pool(name="xb", bufs=4) as xbpool, \
             tc.tile_pool(name="xr", bufs=2) as xrpool, \
             tc.tile_pool(name="qt", bufs=2) as qtpool, \
             tc.tile_pool(name="kt", bufs=2) as ktpool, \
             tc.tile_pool(name="vg", bufs=4) as vgpool, \
             tc.tile_pool(name="at", bufs=8) as atpool, \
             tc.tile_pool(name="yc", bufs=4) as ycpool, \
             tc.tile_pool(name="rr", bufs=4) as rrpool, \
             tc.tile_pool(name="rb", bufs=4) as rbpool, \
             tc.tile_pool(name="ysb", bufs=5) as ysbpool, \
             tc.tile_pool(name="osb", bufs=3) as opool:

            # ---- constants ----
            wqk_t = cpool.tile([C, C], f16, tag="wqk")
            wv_t = cpool.tile([C, CI], f16, tag="wv")
            wz2_t = cpool.tile([C, C], bf16, tag="wz2")
            bq_t = cpool.tile([CI, 1], f32, tag="bq")
            id_t = cpool.tile([C, C], f16, tag="id")

            xb_t = [xbpool.tile([C, HW], f16, tag="xb", name=f"xb{s}")
                    for s in range(4)]
            xr_t = [xrpool.tile([C, HW], f16, tag="xr", name=f"xr{s}")
                    for s in range(NLOC)]
            # ktpair[p]: partitions 0:64 = k of slice p, 64:128 = slice p+2
            kt_t = [ktpool.tile([C, HW], f16, tag="kt", name=f"ktp{p}")
                    for p in range(2)]
            # qt2[n]: q of local slice n duplicated into both halves
            qt_t = [qtpool.tile([C, HW], f16, tag="qt", name=f"qt{n}")
                    for n in range(NLOC)]
            vg_t = [vgpool.tile([C, KC, VW], bf16, tag="vg", name=f"vg{s}")
                    for s in range(4)]

            # ---- load order: projection critical path first ----
            nc.sync.dma_start(out=wqk_t, in_=wqk_d[:, :])
            nc.sync.dma_start(out=xb_t[0], in_=xh_d[0])
            nc.sync.dma_start(out=wv_t, in_=wv_d[:, :])
            nc.sync.dma_start(out=bq_t, in_=bq_d[:, :])
            nc.sync.dma_start(out=vg_t[0][:, :, 0:CI], in_=vgp_d[:, :, :])
            nc.sync.dma_start(out=xb_t[1], in_=xh_d[1])
            nc.sync.dma_start(out=vg_t[1][:, :, 0:CI], in_=vgp_d[:, :, :])
            nc.sync.dma_start(out=xb_t[2], in_=xh_d[2])
            nc.sync.dma_start(out=vg_t[2][:, :, 0:CI], in_=vgp_d[:, :, :])
            nc.sync.dma_start(out=xb_t[3], in_=xh_d[3])
            nc.sync.dma_start(out=vg_t[3][:, :, 0:CI], in_=vgp_d[:, :, :])
            nc.sync.dma_start(out=wz2_t, in_=wz2_d[:, :])
            nc.sync.dma_start(out=id_t, in_=ident_d[:, :])
            for n in range(NLOC):
                nc.sync.dma_start(out=xr_t[n], in_=xr_d[n])

            # ---- projections; evacuations ride casting DMAs (gpsimd swdge)
            # kt bias bk cancels in softmax -> pure cast; q keeps bias bq
            # (ACT) and is duplicated to partitions 64:128 via SBUF DMA.
            T3 = HW // 3             # 768
            KH = KC // 2
            with tc.tile_pool(name="pp", bufs=2, space="PSUM") as pp, \
                 tc.tile_pool(name="pv", bufs=2, space="PSUM") as pv:
                for s in range(4):
                    # kt destination: pair p = s % 2, half = s // 2
                    ktp = kt_t[s % 2]
                    klo = CI * (s // 2)
                    vg = vg_t[s]
                    pvt_h = [None, None]

                    def v_chunk(j, s=s, vg=vg, pvt_h=pvt_h):
                        h, jj = divmod(j, KH)
                        if jj == 0:
                            pvt_h[h] = pv.tile([C, KH * CI], f32, tag="pv",
                                               name=f"pvt{h}")
                        nc.tensor.matmul(
                            pvt_h[h][:, CI * jj:CI * (jj + 1)],
                            lhsT=xb_t[s][:, 128 * j:128 * (j + 1)],
                            rhs=wv_t, start=True, stop=True)
                        if jj == KH - 1:
                            dst = vg[:, KH * h:KH * (h + 1), CI:2 * CI]
                            rsrc = pvt_h[h].rearrange("p (j d) -> p j d",
                                                      d=CI)
                            if h == 0:
                                nc.scalar.activation(dst, rsrc, FT.Copy)
                            else:
                                nc.vector.tensor_copy(dst, rsrc)

                    vidx = 0
                    for t in range(3):
                        t0 = T3 * t
                        pq = pp.tile([C, T3], f32, tag="pp")
                        for (b0, bw) in ((0, 512), (512, 256)):
                            nc.tensor.matmul(pq[:, b0:b0 + bw], lhsT=wqk_t,
                                             rhs=xb_t[s][:, t0 + b0:t0 + b0 + bw],
                                             start=True, stop=True)
                        for _ in range(6):
                            v_chunk(vidx)
                            vidx += 1
                        # k: cast-only copy into the packed pair tile
                        # (bias bk cancels in softmax); split ACT/DVE
                        kdst = ktp[klo:klo + CI, t0:t0 + T3]
                        if (3 * s + t) % 4 == 0:
                            nc.scalar.activation(kdst, pq[CI:C, :], FT.Copy)
                        else:
                            nc.vector.tensor_copy(kdst, pq[CI:C, :])
                        if s in (1, 2):
                            qt = qt_t[s - 1]
                            nc.scalar.activation(qt[0:CI, t0:t0 + T3],
                                                 pq[0:CI, :], FT.Identity,
                                                 bias=bq_t)
                            nc.sync.dma_start(
                                out=qt[CI:C, t0:t0 + T3],
                                in_=qt[0:CI, t0:t0 + T3])

            # ---- attention ----
            # PSUM: 3 pair tiles ([128,1024] = 2 banks each; fA in bank 0,
            # fB in bank 1, exp reads the contiguous span) + 2 yps banks.
            # zps borrows a pair-pool slot.  Separate pool tensors keep the
            # dependency tracker precise (strided APs over one big tensor
            # go conservative and serialize the PE stream).
            with tc.tile_pool(name="pf", bufs=3, space="PSUM") as pf, \
                 tc.tile_pool(name="py", bufs=2, space="PSUM") as py:

                units = [(n, qi) for n in range(NLOC)
                         for qi in range(len(QBS))]
                pend = {}          # slot_idx -> [fn]
                slotbox = [0]
                eidx = [0]

                def fire_pend():
                    s = slotbox[0]
                    if s in pend:
                        for fn in pend.pop(s):
                            fn()

                def sched(ds, fn):
                    pend.setdefault(slotbox[0] + ds, []).append(fn)

                for ui, (n, qi) in enumerate(units):
                    q0, w = QBS[qi]
                    st = {"yps": [None, None], "ft": {}, "at": {}}

                    def fpair(j, n=n, q0=q0, w=w, st=st):
                        ft = pf.tile([C, 1024], f32, tag="ft", name="ftp")
                        st["ft"][j] = ft
                        nc.tensor.matmul(
                            ft[:, 0:w],
                            lhsT=kt_t[n][0:CI, 128 * j:128 * (j + 1)],
                            rhs=qt_t[n][0:CI, q0:q0 + w],
                            start=True, stop=True)
                        nc.tensor.matmul(
                            ft[:, BW:BW + w],
                            lhsT=kt_t[n][CI:C, 128 * j:128 * (j + 1)],
                            rhs=qt_t[n][CI:C, q0:q0 + w],
                            start=True, stop=True)

                    def expf(j, st=st, w=w):
                        ft = st["ft"].pop(j)
                        span = BW + w
                        # allocate as i16 and bitcast on the read side:
                        # a bitcast WRITE AP defeats precise dependency
                        # tracking and the DVE exp then barriers on the
                        # whole PE stream
                        ati = atpool.tile([C, 1024], i16, tag="at",
                                          name="atp")
                        at = ati.bitcast(bf16)
                        # 10 exps on ACT (true exp), 8 on DVE (fast-exp)
                        if j % 2 == 0 or j == KC - 1:
                            nc.scalar.activation(at[:, 0:span],
                                                 ft[:, 0:span], FT.Exp)
                        else:
                            nc.vector.tensor_scalar(
                                ati[:, 0:span], ft[:, 0:span],
                                _S16, _B16,
                                op0=mybir.AluOpType.mult,
                                op1=mybir.AluOpType.add)
                        st["at"][j] = at

                    def ypair(j, n=n, w=w, st=st):
                        first = (j == 0)
                        last = (j == KC - 1)
                        if first:
                            for side in range(2):
                                yt = py.tile([C, BW], f32, tag="py",
                                             name=f"yps{side}")
                                st["yps"][side] = yt
                        at = st["at"].pop(j)
                        for side in range(2):
                            kv = n + 2 * side
                            nc.tensor.matmul(
                                st["yps"][side][:, 0:w],
                                lhsT=vg_t[kv][:, j, :],
                                rhs=at[:, BW * side:BW * side + w],
                                start=first, stop=last)

                    # ---- tail ops for THIS unit, deferred ----
                    yc = [None, None]
                    rr = [None, None]
                    rb = [None, None]
                    ysb = [None]
                    zbox = [None]

                    def yc_copy(side, st=st, w=w, yc=yc):
                        t = ycpool.tile([C, BW], f32, tag="yc",
                                        name=f"yc{side}")
                        yc[side] = t
                        if side == 0:
                            nc.scalar.activation(
                                t[:, 0:w], st["yps"][side][:, 0:w],
                                FT.Copy)
                        else:
                            nc.vector.tensor_copy(
                                t[:, 0:w], st["yps"][side][:, 0:w])

                    def recip(side, w=w, yc=yc, rr=rr):
                        t = rrpool.tile([1, BW], f32, tag="rr",
                                        name=f"rr{side}")
                        rr[side] = t
                        nc.vector.reciprocal_approx_fast(
                            t[:, 0:w], yc[side][0:1, 0:w])

                    def bcast(side, w=w, rr=rr, rb=rb):
                        t = rbpool.tile([C, BW], f32, tag="rb",
                                        name=f"rb{side}")
                        rb[side] = t
                        nc.gpsimd.partition_broadcast(t[:, 0:w],
                                                      rr[side][:, 0:w])

                    def mul(side, w=w, yc=yc, rb=rb, ysb=ysb):
                        # all-SBUF on gpsimd; 64-aligned partition shift
                        if ysb[0] is None:
                            ysb[0] = ysbpool.tile([C, BW], bf16,
                                                  tag="ysb", name="ysb")
                        nc.gpsimd.tensor_mul(
                            ysb[0][CI * side:CI * (side + 1), 0:w],
                            yc[side][CI:C, 0:w],
                            rb[side][CI:C, 0:w])

                    def zout(n=n, q0=q0, w=w, ysb=ysb, zbox=zbox):
                        zt = pf.tile([C, 1024], f32, tag="ft", name="zps")
                        zps = zt[:, 0:w]
                        nc.tensor.matmul(zps, lhsT=wz2_t,
                                         rhs=ysb[0][:, 0:w],
                                         start=True, stop=False)
                        nc.tensor.matmul(zps, lhsT=id_t,
                                         rhs=xr_t[n][:, q0:q0 + w],
                                         start=False, stop=True)
                        # DMA can't read PSUM: split-column evac on
                        # DVE+ACT so the z slot frees fast, then DMA
                        osb = opool.tile([C, BW], f32, tag="osb",
                                         name="osb")
                        h2 = w // 2
                        nc.vector.tensor_copy(osb[:, 0:h2], zt[:, 0:h2])
                        nc.scalar.activation(osb[:, h2:w], zt[:, h2:w],
                                             FT.Copy)
                        nc.sync.dma_start(out=out_d[n][:, q0:q0 + w],
                                          in_=osb[:, 0:w])

                    # ---- emit this unit's slots (y lags f by 3 slots so
                    # the merged exp latency is fully covered) ----
                    for j in range(KC):
                        fpair(j)
                        expf(j)
                        fire_pend()
                        if j >= 3:
                            ypair(j - 3)
                        slotbox[0] += 1

                    # cross-boundary: last three y pairs + evacuations
                    # (bind fns via defaults -- loop scope names rebind!)
                    sched(0, lambda f=ypair: f(KC - 3))
                    sched(1, lambda f=ypair: f(KC - 2))
                    sched(2, lambda f=ypair: f(KC - 1))
                    sched(3, lambda f=yc_copy: f(0))
                    sched(3, lambda f=yc_copy: f(1))
                    sched(5, lambda f=recip: f(0))
                    sched(6, lambda f=recip: f(1))
                    sched(8, lambda f=bcast: f(0))
                    sched(9, lambda f=bcast: f(1))
                    sched(11, lambda f=mul: f(0))
                    sched(12, lambda f=mul: f(1))
                    # z flushes THREE units later: its matmul sits in the
                    # strict-FIFO PE queue, so it must never wait on the
                    # (engine-hopping) ysb chain or it blocks the stream
                    sched(52, zout)

                # drain remaining pend slots (no more f work)
                while pend:
                    fire_pend()
                    slotbox[0] += 1

    nc.compile()
    return nc


def _get_nc():
    if "nc" not in _NC_CACHE:
        _NC_CACHE["nc"] = _build_nc()
    return _NC_CACHE["nc"]


def _host_inputs(features, Wq, bq, Wk, bk, Wv, bv, Wz, bz):
    import ml_dtypes
    X = np.ascontiguousarray(
        np.asarray(features, np.float32).reshape(N_FULL, C, HW))
    wqk = np.ascontiguousarray(
        np.concatenate([Wq.T, Wk.T], axis=1).astype(np.float16))
    wv = np.ascontiguousarray(np.asarray(Wv).T.astype(np.float16))
    bqv = np.asarray(bq, np.float32).reshape(CI, 1)
    c2 = (2.0 * (np.asarray(Wz) @ np.asarray(bv) + np.asarray(bz))).astype(
        np.float32)
    ident = np.eye(C, dtype=np.float16)
    wzT = np.asarray(Wz).T.astype(np.float32)
    wz2 = np.ascontiguousarray(
        np.concatenate([wzT, wzT], axis=0).astype(ml_dtypes.bfloat16))
    vgp = np.zeros((C, KC, CI), ml_dtypes.bfloat16)
    vgp[:, :, 0] = 1.0
    in_maps = []
    for i in range(NCORES):
        idx = [max(2 * i - 1, 0), 2 * i, 2 * i + 1, min(2 * i + 2, N_FULL - 1)]
        xr = (X[2 * i:2 * i + 2] + c2[None, :, None]).astype(np.float16)
        in_maps.append({
            "xh": np.ascontiguousarray(X[idx].astype(np.float16)),
            "xr": np.ascontiguousarray(xr),
            "wqk": wqk, "wv": wv, "wz2": wz2, "bq": bqv,
            "ident": ident, "vgp": vgp,
        })
    return in_maps


def kernel(features, Wq, bq, Wk, bk, Wv, bv, Wz, bz):
    global LAST_RESULTS
    from concourse.bass_utils import run_bass_kernel_spmd

    nc = _get_nc()
    in_maps = _host_inputs(features, Wq, bq, Wk, bk, Wv, bv, Wz, bz)
    res = run_bass_kernel_spmd(nc, in_maps, core_ids=list(range(NCORES)),
                               trace=TRACE)
    LAST_RESULTS = res
    out = np.empty((N_FULL, C, H, W), np.float32)
    for i in range(NCORES):
        out[2 * i:2 * i + 2] = res.results[i]["out"].reshape(NLOC, C, H, W)
    return out
